# revision 1
# baseline (speedup 1.0000x reference)
"""Trainium2 Bass kernel for nn_AdjacencyMatrixLayer.

Computes, per batch sample b (coordinates x in R^{N x 3}):
    d_ij  = |x_i|^2 - 2 x_i.x_j + |x_j|^2
    A     = exp(-d / sigma^2)
    A     = softmax(A, axis=2) * mask
    out   = A / (sum_j A + 1e-20)

Device-side restructuring:
  * One K=22 bf16 matmul produces y = -d/sigma^2 + C*(v_i*v_j - 1):
    the first 20 rows are the hi/lo-split augmented coordinates (exact
    to ~2^-18); the last 2 rows fold the padding mask into the PE
    (C=144, v = 0/1 valid bits), so masked entries get y - 144 and the
    mask tensor is never shipped or multiplied.
  * The double exponential F(y) = exp(exp(y)) is approximated, up to a
    global per-row-cancelling scale K, by a quadratic in p = exp(s*y):
        q = p^2 + a*p + b ~= K * exp(exp(y)),  minimax rel err 5.1e-3
    (cubic variant: q = p^3+a p^2+b p+c, rel err 3.2e-4, one more DVE
    pass).  Masked entries give p = 0 exactly (exp underflow), q = b,
    and contribute b per element to the row sum, so
        sum_valid q = accum(t) + b*L_b
    with L_b the per-sample valid length — a host-provided constant.
    Softmax renormalization makes K and the final 1/sum exact:
        out = q / sum_valid(q)   on the valid [L,L] block
    and the host zero-fills the masked region of the output.
  * Engine placement per [128, 2048] row-block:
      PE : 4 bf16 matmuls (512-col PSUM banks)        ~1.9us
      ACT: p = Exp(scale*psum), fp16 out              ~2.0us
      DVE: t = (p + a) * p, accum -> qs   (stt is 1x on TRN2 HW: 2.27us;
           TENSOR_SCALAR is the only 4x op, and needs one tensor input)
      Pool: qs2 = qs + b*L (tiny) ; DVE: r = 1/qs2 (tiny)
      scale pass out = (t + b) * r: DVE tensor_scalar (4x, 0.75us) for
           19/32 blocks, ACT Identity(t*r + b*r) for 13/32 blocks —
           balances ACT ~90us vs DVE ~90us instead of DVE-bound 98us
      DMA: out row-block, alternating SWDGE (gpsimd) / HWDGE (sync)
  * Measured per-core: span ~113.5us, scalar/vector ~90us each,
    tensor ~61us, dma ~53us.  (baseline 144.6us; rel err 1.04e-2)
"""

import sys

import numpy as np

for _p in ("/opt/trn_rl_repo", "/root/.axon_site/_ro/trn_rl_repo"):
    if _p not in sys.path:
        sys.path.append(_p)

B, N, D = 16, 2048, 3
NCORES = 8
SPC = B // NCORES  # samples per core
P = 128            # SBUF partitions
MMF = 512          # matmul moving free-dim chunk (= 1 PSUM bank of fp32)
NB = SPC * N // P  # row-blocks per core
KAUG = 22          # 20 hi/lo aug rows + 2 mask-fold rows
MASKC = 144.0      # mask fold offset: masked entries get y - 144

MODE = "quad"      # "quad" (2 DVE passes) or "cubic" (3 DVE passes)
POOL_STT = 0       # Pool Q7 firmware lacks stt/ts; keep all stt on DVE
POOL_ACCUM = True  # Pool stt supports accum_out (fallback: extra DVE reduce)
ACT_SCALE = 13     # final-scale passes moved to ACT Copy (out = t*r + b*r)

# minimax fits of (poly in p) / (K * exp(exp(y))) - 1 over y <= 0
QS_S = 0.9943403856229558   # p = exp(QS_S * y)
QS_A = 1.05888673672267     # q = p^2 + QS_A*p + QS_B
QS_B = 1.217950642291432
CU_A = 1.600139700859946    # q = p^3 + CU_A*p^2 + CU_B*p + CU_C, p = exp(y)
CU_B = 3.7300379796011542
CU_C = 3.6840145818755072

_CACHE: dict = {}


def _build(mode):
    import concourse.bacc as bacc
    import concourse.tile as tile
    from concourse import mybir

    f32 = mybir.dt.float32
    f16 = mybir.dt.float16
    bf16 = mybir.dt.bfloat16
    AT = mybir.AluOpType
    nc = bacc.Bacc(None, target_bir_lowering=False, debug=False)

    aug_x = nc.dram_tensor("aug_x", [SPC, KAUG, N], bf16, kind="ExternalInput")
    aug_y = nc.dram_tensor("aug_y", [SPC, KAUG, N], bf16, kind="ExternalInput")
    # per-sample row-sum correction, replicated across partitions:
    # quad: b*L ; cubic: c*L
    cons = nc.dram_tensor("cons", [SPC, P, 1], f32, kind="ExternalInput")
    out = nc.dram_tensor("out", [SPC, N, N], f16, kind="ExternalOutput")

    o_flat = out.rearrange("s n m -> (s n) m")

    with tile.TileContext(nc) as tc:
        with (
            tc.tile_pool(name="consts", bufs=1) as consts,
            tc.tile_pool(name="work", bufs=10) as workp,
            tc.tile_pool(name="ot", bufs=8) as otp,
            tc.tile_pool(name="small", bufs=16) as smallp,
            tc.tile_pool(name="psum", bufs=2, space="PSUM") as psump,
        ):
            augx_t, augy_t, cons_t = [], [], []
            for s in range(SPC):
                ax = consts.tile([KAUG, N], bf16, tag=f"augx{s}")
                ay = consts.tile([KAUG, N], bf16, tag=f"augy{s}")
                cn = consts.tile([P, 1], f32, tag=f"cons{s}")
                nc.sync.dma_start(out=ax, in_=aug_x[s])
                nc.gpsimd.dma_start(out=ay, in_=aug_y[s])
                nc.sync.dma_start(out=cn, in_=cons[s])
                augx_t.append(ax)
                augy_t.append(ay)
                cons_t.append(cn)

            npool = 0
            for ib in range(NB):
                s = ib // (N // P)
                i0 = (ib % (N // P)) * P

                ps = psump.tile([P, N], f32)
                for j in range(N // MMF):
                    nc.tensor.matmul(
                        ps[:, j * MMF:(j + 1) * MMF],
                        augx_t[s][:, i0:i0 + P],
                        augy_t[s][:, j * MMF:(j + 1) * MMF],
                    )

                # Bresenham spread of POOL_STT pool-blocks over NB
                use_pool = ((ib + 1) * POOL_STT // NB) != (ib * POOL_STT // NB)
                npool += use_pool
                stt_eng = nc.gpsimd if use_pool else nc.vector

                p = workp.tile([P, N], f16, tag="p")
                qs = smallp.tile([P, 1], f32, tag="qs")
                if mode == "quad":
                    # p = exp(s*y); t = (p + a)*p ; qs = sum_j t
                    nc.scalar.activation(
                        p, ps, mybir.ActivationFunctionType.Exp, scale=QS_S
                    )
                    t = workp.tile([P, N], f16, tag="t")
                    if use_pool and not POOL_ACCUM:
                        stt_eng.scalar_tensor_tensor(
                            out=t, in0=p, scalar=QS_A, in1=p,
                            op0=AT.add, op1=AT.mult,
                        )
                        # row sum via a cheap DVE 4x pass into a scratch tile
                        tsc = workp.tile([P, N], f16, tag="tsc")
                        nc.vector.tensor_scalar(
                            out=tsc, in0=t, scalar1=1.0, scalar2=None,
                            op0=AT.mult, accum_out=qs,
                        )
                    else:
                        stt_eng.scalar_tensor_tensor(
                            out=t, in0=p, scalar=QS_A, in1=p,
                            op0=AT.add, op1=AT.mult, accum_out=qs,
                        )
                    cfin = QS_B
                else:
                    # p = exp(y); t1 = (p + a)*p ; t = (t1 + b)*p ; qs = sum t
                    nc.scalar.activation(p, ps, mybir.ActivationFunctionType.Exp)
                    t1 = workp.tile([P, N], f16, tag="t1")
                    stt_eng.scalar_tensor_tensor(
                        out=t1, in0=p, scalar=CU_A, in1=p,
                        op0=AT.add, op1=AT.mult,
                    )
                    t = workp.tile([P, N], f16, tag="t")
                    stt_eng.scalar_tensor_tensor(
                        out=t, in0=t1, scalar=CU_B, in1=p,
                        op0=AT.add, op1=AT.mult, accum_out=qs,
                    )
                    cfin = CU_C

                # qs2 = qs + const*L ; r = 1/qs2   (tiny [P,1] ops on DVE)
                qs2 = smallp.tile([P, 1], f32, tag="qs2")
                nc.gpsimd.tensor_tensor(
                    out=qs2, in0=qs, in1=cons_t[s], op=AT.add
                )
                r = smallp.tile([P, 1], f32, tag="r")
                nc.vector.reciprocal(r, qs2)

                # out = (t + cfin) * r
                ot = otp.tile([P, N], f16, tag="ot")
                use_act = ((ib + 1) * ACT_SCALE // NB) != (ib * ACT_SCALE // NB)
                if use_act:
                    # ACT Copy computes t*scale + bias with [P,1] APs
                    br = smallp.tile([P, 1], f32, tag="br")
                    nc.vector.tensor_scalar_mul(br, r, cfin)
                    nc.scalar.activation(
                        ot, t, mybir.ActivationFunctionType.Identity,
                        scale=r, bias=br,
                    )
                else:
                    nc.vector.tensor_scalar(
                        out=ot, in0=t, scalar1=cfin, scalar2=r,
                        op0=AT.add, op1=AT.mult,
                    )
                out_eng = nc.gpsimd if ib % 2 == 0 else nc.sync
                nc_eng = out_eng
                nc_eng.dma_start(out=o_flat[ib * P:(ib + 1) * P, :], in_=ot)

    nc.compile()
    return nc


def _lengths_from_masks(masks):
    """Per-sample valid lengths; verifies the product-prefix structure."""
    diag = np.einsum('bii->bi', masks)
    valid = (diag > 0.5).astype(np.float32)
    lengths = valid.sum(axis=1).astype(np.int64)
    # prefix check + product check (cheap, exact)
    n = masks.shape[1]
    pref = (np.arange(n)[None, :] < lengths[:, None]).astype(np.float32)
    if not np.array_equal(valid, pref):
        return None
    if not np.array_equal(masks, valid[:, :, None] * valid[:, None, :]):
        return None
    return lengths, valid


def _prepare(coordinates, masks, sigma):
    """Host-side prep: shard over cores, build augmented coordinates."""
    import ml_dtypes

    bf = ml_dtypes.bfloat16
    coords = np.ascontiguousarray(np.asarray(coordinates, dtype=np.float32))
    masks = np.asarray(masks, dtype=np.float32)
    sig = float(np.asarray(sigma, dtype=np.float32).reshape(-1)[0])

    res = _lengths_from_masks(masks)
    assert res is not None, "masks are not product-of-prefix form"
    lengths, valid = res

    norms = np.sum(coords * coords, axis=2, dtype=np.float32)  # [B, N]
    xT = np.swapaxes(coords, 1, 2)                             # [B, 3, N]
    nss = np.float32(-1.0 / (sig * sig))
    aug_x = np.empty((B, 5, N), np.float32)
    aug_x[:, 0:3] = (-2.0 * nss) * xT
    aug_x[:, 3] = nss * norms
    aug_x[:, 4] = nss
    aug_y = np.empty((B, 5, N), np.float32)
    aug_y[:, 0:3] = xT
    aug_y[:, 3] = 1.0
    aug_y[:, 4] = norms

    # hi/lo bf16 split: v = hi + lo, K=5 fp32 -> K=20 bf16 contraction
    xh = aug_x.astype(bf)
    xl = (aug_x - xh.astype(np.float32)).astype(bf)
    yh = aug_y.astype(bf)
    yl = (aug_y - yh.astype(np.float32)).astype(bf)
    # mask fold rows: C*v_i*v_j - C  (exact in bf16: C=144, v in {0,1})
    C = np.float32(MASKC)
    mx = np.stack([C * valid, np.full_like(valid, C)], axis=1).astype(bf)
    my = np.stack([valid, np.full_like(valid, -1.0)], axis=1).astype(bf)
    augx22 = np.concatenate([xh, xl, xh, xl, mx], axis=1)  # [B, 22, N]
    augy22 = np.concatenate([yh, yh, yl, yl, my], axis=1)

    ccoef = QS_B if MODE == "quad" else CU_C
    consv = (np.float32(ccoef) * lengths.astype(np.float32))  # [B]
    cons = np.broadcast_to(consv[:, None, None], (B, P, 1)).astype(np.float32)

    in_maps = []
    for c in range(NCORES):
        lo, hi = c * SPC, (c + 1) * SPC
        in_maps.append({
            "aug_x": np.ascontiguousarray(augx22[lo:hi]),
            "aug_y": np.ascontiguousarray(augy22[lo:hi]),
            "cons": np.ascontiguousarray(cons[lo:hi]),
        })
    return in_maps, lengths


def _get_nc():
    if "nc" not in _CACHE:
        _CACHE["nc"] = _build(MODE)
    return _CACHE["nc"]


def kernel(coordinates, masks, sigma):
    import time

    from concourse.bass_utils import run_bass_kernel_spmd

    in_maps, lengths = _prepare(coordinates, masks, sigma)
    # the shared trn2 device occasionally reports a transient
    # NRT_EXEC_UNIT_UNRECOVERABLE; it clears on its own within ~a minute
    for attempt in range(4):
        try:
            res = run_bass_kernel_spmd(
                _get_nc(), in_maps, core_ids=list(range(NCORES))
            )
            break
        except Exception:  # noqa: BLE001 - retry transient device errors
            if attempt == 3:
                raise
            time.sleep(20 * (attempt + 1))

    full = np.zeros((B, N, N), np.float32)
    for b in range(B):
        c, s = b // SPC, b % SPC
        L = int(lengths[b])
        full[b, :L, :L] = res.results[c]["out"][s, :L, :L].astype(np.float32)
    return full



# revision 7
# speedup vs baseline: 1.3834x; 1.3834x over previous
"""Trainium2 Bass kernel for nn_AdjacencyMatrixLayer.

Computes, per batch sample b (coordinates x in R^{N x 3}):
    d_ij  = |x_i|^2 - 2 x_i.x_j + |x_j|^2
    A     = exp(-d / sigma^2)
    A     = softmax(A, axis=2) * mask
    out   = A / (sum_j A + 1e-20)

Key structural ideas (v2, on top of the v1 quad kernel):
  * Valid-region truncation: masks are product-of-prefix (valid lengths
    L_b in [N/2, N]); out is zero outside [:L,:L].  Only row-blocks with
    rows < L are computed, at column width W = ceil(L/128)*128, cutting
    ~45% of all engine + DMA work (sum L^2 / (B*N^2) ~ 0.51).
  * Block-major SPMD packing: the work unit is a [128, W] row-block.
    All 8 cores execute ONE identical width-schedule (widths padded so
    each bucket count is divisible by 8); which (sample, row-range) a
    block holds is pure per-core DATA (stationary/moving slices packed
    host-side), so load balance is near-perfect regardless of lengths.
  * One K=22 bf16 matmul per block produces y = -d/sigma^2 - C*(1-v_i*v_j):
    20 hi/lo-split augmented coordinate rows (exact to ~2^-18) + 2 rows
    folding the padding mask (C=144), so masked entries get y - 144.
  * Per block, one of two pointwise schemes, greedily mixed to balance
    the scalar (ACT) and vector (DVE) engines:
      Q (quad):  p = Exp(s*y) on ACT; t = (p+a)*p + accum on DVE stt
                 (1x); out = (t+b)*r on DVE ts (4x).  Minimax quadratic
                 q = p^2+a*p+b ~= K*exp(exp(y)), rel err 5.1e-3.
      E (exact): A = Exp(y) on ACT; q = Exp(A) + accum on ACT;
                 out = q*r on DVE ts (4x).  Exact double exponential;
                 masked entries give A=0, q=1, corrected via the
                 host-provided per-block constant.
    Row renormalization r = 1/(accum + cons) makes the overall scale
    exact; host zero-fills outside [:L,:L].
  * All inputs preloaded in 3 DMAs; output DMA alternates SWDGE/HWDGE.
"""

import math
import sys

import numpy as np

for _p in ("/opt/trn_rl_repo", "/root/.axon_site/_ro/trn_rl_repo"):
    if _p not in sys.path:
        sys.path.append(_p)

B, N, D = 16, 2048, 3
NCORES = 8
P = 128            # SBUF partitions / rows per block
MMF = 512          # matmul moving free-dim chunk (= 1 PSUM bank of fp32)
KAUG = 22          # 20 hi/lo aug rows + 2 mask-fold rows
MASKC = 144.0      # mask fold offset: masked entries get y - 144

# minimax fit of (p^2 + a*p + b) / (K * exp(exp(y))) - 1 over y <= 0
QS_S = 0.9943403856229558   # p = exp(QS_S * y)
QS_A = 1.05888673672267     # q = p^2 + QS_A*p + QS_B
QS_B = 1.217950642291432

# engine-time model (ns per moving column) for the ACT/DVE balance greedy
ACT_NS = 0.977     # one ACT pass over [128, W]
DVE_STT_NS = 1.108  # DVE scalar_tensor_tensor (1x)
DVE_TS_NS = 0.366   # DVE tensor_scalar (4x)
DVE_FIX_NS = 300.0  # per-block small ops on DVE (reciprocal etc)

_CACHE: dict = {}


def _schedule(lengths):
    """Build the common width schedule + per-core block assignment.

    Returns (widths, schemes, assign) where widths[k] is slot k's moving
    width (same for every core), schemes[k] in {"Q", "E"}, and
    assign[c][k] = (sample, row0, width) for core c slot k (dummy slots
    duplicate a real block; their output is ignored).
    """
    blocks = []  # (natural_width, sample, row0)
    for b, L in enumerate(lengths):
        nb = (int(L) + P - 1) // P
        w = nb * P
        for r in range(nb):
            blocks.append((w, b, r * P))

    buckets = sorted({w for w, _, _ in blocks}, reverse=True)
    byb = {w: [] for w in buckets}
    for blk in blocks:
        byb[blk[0]].append(blk)

    # promote blocks upward so every bucket count is divisible by NCORES
    for i, w in enumerate(buckets):
        need = (-len(byb[w])) % NCORES
        j = i + 1
        while need and j < len(buckets):
            take = byb[buckets[j]][:need]
            byb[buckets[j]] = byb[buckets[j]][need:]
            byb[w].extend(take)
            need -= len(take)
            j += 1
        while need:  # bottom bucket: pad with dummy duplicates
            byb[w].append((byb[w][0][0], byb[w][0][1], byb[w][0][2], "dummy"))
            need -= 1

    widths, assign = [], [[] for _ in range(NCORES)]
    for w in buckets:
        blks = byb[w]
        if not blks:
            continue
        npc = len(blks) // NCORES
        widths.extend([w] * npc)
        for c in range(NCORES):
            for blk in blks[c * npc:(c + 1) * npc]:
                assign[c].append((blk[1], blk[2], w))

    # greedy ACT/DVE balance: scheme per slot
    schemes = []
    act_t = dve_t = 0.0
    for w in widths:
        aq = act_t + ACT_NS * w
        dq = dve_t + (DVE_STT_NS + DVE_TS_NS) * w + DVE_FIX_NS
        ae = act_t + 2 * ACT_NS * w
        de = dve_t + DVE_TS_NS * w + DVE_FIX_NS
        if max(ae, de) < max(aq, dq):
            schemes.append("E")
            act_t, dve_t = ae, de
        else:
            schemes.append("Q")
            act_t, dve_t = aq, dq
    return widths, schemes, assign


def _build(widths, schemes):
    import concourse.bacc as bacc
    import concourse.tile as tile
    from concourse import mybir

    f32 = mybir.dt.float32
    f16 = mybir.dt.float16
    bf16 = mybir.dt.bfloat16
    AT = mybir.AluOpType
    AF = mybir.ActivationFunctionType
    nc = bacc.Bacc(None, target_bir_lowering=False, debug=False)

    nblk = len(widths)
    sumw = sum(widths)
    offs = np.concatenate([[0], np.cumsum(widths)]).tolist()

    stat = nc.dram_tensor("stat", [KAUG, nblk * P], bf16, kind="ExternalInput")
    mov = nc.dram_tensor("mov", [KAUG, sumw], bf16, kind="ExternalInput")
    cons = nc.dram_tensor("cons", [P, nblk], f32, kind="ExternalInput")
    out = nc.dram_tensor("out", [nblk * P, N], f16, kind="ExternalOutput")

    with tile.TileContext(nc) as tc:
        with (
            tc.tile_pool(name="consts", bufs=1) as consts,
            tc.tile_pool(name="work", bufs=10) as workp,
            tc.tile_pool(name="ot", bufs=8) as otp,
            tc.tile_pool(name="small", bufs=16) as smallp,
            tc.tile_pool(name="psum", bufs=2, space="PSUM") as psump,
        ):
            mv_all = consts.tile([KAUG, sumw], bf16, tag="mv")
            st_all = consts.tile([KAUG, nblk * P], bf16, tag="st")
            cn_all = consts.tile([P, nblk], f32, tag="cn")
            nc.sync.dma_start(out=mv_all, in_=mov[:, :])
            nc.gpsimd.dma_start(out=st_all, in_=stat[:, :])
            nc.sync.dma_start(out=cn_all, in_=cons[:, :])

            swdge_bytes = hwdge_bytes = 0
            for k in range(nblk):
                W = widths[k]
                st = st_all[:, k * P:(k + 1) * P]
                mv = mv_all[:, offs[k]:offs[k] + W]

                ps = psump.tile([P, N], f32)
                for c0 in range(0, W, MMF):
                    cw = min(MMF, W - c0)
                    nc.tensor.matmul(
                        ps[:, c0:c0 + cw], st, mv[:, c0:c0 + cw]
                    )

                qs = smallp.tile([P, 1], f32, tag="qs")
                t = workp.tile([P, N], f16, tag="t")
                if schemes[k] == "Q":
                    # p = exp(s*y); t = (p + a)*p ; qs = sum_j t
                    p = workp.tile([P, N], f16, tag="p")
                    nc.scalar.activation(
                        p[:, :W], ps[:, :W], AF.Exp, scale=QS_S
                    )
                    nc.vector.scalar_tensor_tensor(
                        out=t[:, :W], in0=p[:, :W], scalar=QS_A, in1=p[:, :W],
                        op0=AT.add, op1=AT.mult, accum_out=qs,
                    )
                    cfin = QS_B
                else:
                    # A = exp(y); t = exp(A) ; qs = sum_j t   (exact)
                    a_t = workp.tile([P, N], f16, tag="p")
                    nc.scalar.activation(a_t[:, :W], ps[:, :W], AF.Exp)
                    nc.scalar.activation(
                        t[:, :W], a_t[:, :W], AF.Exp, accum_out=qs
                    )
                    cfin = 0.0

                # qs2 = qs + cons ; r = 1/qs2   (tiny [P,1] ops)
                qs2 = smallp.tile([P, 1], f32, tag="qs2")
                nc.gpsimd.tensor_tensor(
                    out=qs2, in0=qs, in1=cn_all[:, k:k + 1], op=AT.add
                )
                r = smallp.tile([P, 1], f32, tag="r")
                nc.vector.reciprocal(r, qs2)

                # out = (t + cfin) * r   on DVE ts (4x)
                ot = otp.tile([P, N], f16, tag="ot")
                nc.vector.tensor_scalar(
                    out=ot[:, :W], in0=t[:, :W], scalar1=cfin, scalar2=r,
                    op0=AT.add, op1=AT.mult,
                )
                if swdge_bytes <= hwdge_bytes:
                    swdge_bytes += W * P * 2
                    out_eng = nc.gpsimd
                else:
                    hwdge_bytes += W * P * 2
                    out_eng = nc.sync
                out_eng.dma_start(
                    out=out[k * P:(k + 1) * P, :W], in_=ot[:, :W]
                )

    nc.compile()
    return nc


def _lengths_from_masks(masks):
    """Per-sample valid lengths; verifies the product-prefix structure."""
    diag = np.einsum('bii->bi', masks)
    valid = (diag > 0.5).astype(np.float32)
    lengths = valid.sum(axis=1).astype(np.int64)
    n = masks.shape[1]
    pref = (np.arange(n)[None, :] < lengths[:, None]).astype(np.float32)
    if not np.array_equal(valid, pref):
        return None
    if not np.array_equal(masks, valid[:, :, None] * valid[:, None, :]):
        return None
    return lengths, valid


def _prepare(coordinates, masks, sigma):
    """Host-side prep: schedule blocks, pack per-core block-major inputs."""
    import ml_dtypes

    bf = ml_dtypes.bfloat16
    coords = np.ascontiguousarray(np.asarray(coordinates, dtype=np.float32))
    masks = np.asarray(masks, dtype=np.float32)
    sig = float(np.asarray(sigma, dtype=np.float32).reshape(-1)[0])

    res = _lengths_from_masks(masks)
    assert res is not None, "masks are not product-of-prefix form"
    lengths, valid = res
    widths, schemes, assign = _schedule(lengths)
    nblk = len(widths)
    sumw = sum(widths)
    offs = np.concatenate([[0], np.cumsum(widths)])

    norms = np.sum(coords * coords, axis=2, dtype=np.float32)  # [B, N]
    xT = np.swapaxes(coords, 1, 2)                             # [B, 3, N]
    nss = np.float32(-1.0 / (sig * sig))
    aug_x = np.empty((B, 5, N), np.float32)
    aug_x[:, 0:3] = (-2.0 * nss) * xT
    aug_x[:, 3] = nss * norms
    aug_x[:, 4] = nss
    aug_y = np.empty((B, 5, N), np.float32)
    aug_y[:, 0:3] = xT
    aug_y[:, 3] = 1.0
    aug_y[:, 4] = norms

    # hi/lo bf16 split: v = hi + lo, K=5 fp32 -> K=20 bf16 contraction
    xh = aug_x.astype(bf)
    xl = (aug_x - xh.astype(np.float32)).astype(bf)
    yh = aug_y.astype(bf)
    yl = (aug_y - yh.astype(np.float32)).astype(bf)
    # mask fold rows: C*v_i*v_j - C  (exact in bf16: C=144, v in {0,1})
    C = np.float32(MASKC)
    mx = np.stack([C * valid, np.full_like(valid, C)], axis=1).astype(bf)
    my = np.stack([valid, np.full_like(valid, -1.0)], axis=1).astype(bf)
    augx22 = np.concatenate([xh, xl, xh, xl, mx], axis=1)  # [B, 22, N]
    augy22 = np.concatenate([yh, yh, yl, yl, my], axis=1)

    in_maps = []
    for c in range(NCORES):
        stat = np.empty((KAUG, nblk * P), bf)
        mov = np.empty((KAUG, sumw), bf)
        cons = np.empty((P, nblk), np.float32)
        for k, (b, r0, w) in enumerate(assign[c]):
            stat[:, k * P:(k + 1) * P] = augx22[b][:, r0:r0 + P]
            mov[:, offs[k]:offs[k] + w] = augy22[b][:, :w]
            L = float(lengths[b])
            cons[:, k] = QS_B * L if schemes[k] == "Q" else -(w - L)
        in_maps.append({"stat": stat, "mov": mov, "cons": cons})
    return in_maps, (lengths, widths, schemes, assign)


def _get_nc(widths=None, schemes=None):
    if "nc" not in _CACHE:
        _CACHE["nc"] = _build(widths, schemes)
    return _CACHE["nc"]


def kernel(coordinates, masks, sigma):
    import time

    from concourse.bass_utils import run_bass_kernel_spmd

    in_maps, (lengths, widths, schemes, assign) = _prepare(
        coordinates, masks, sigma
    )
    nc = _get_nc(widths, schemes)
    # the shared trn2 device occasionally reports a transient
    # NRT_EXEC_UNIT_UNRECOVERABLE; it clears on its own within ~a minute
    for attempt in range(4):
        try:
            res = run_bass_kernel_spmd(
                nc, in_maps, core_ids=list(range(NCORES))
            )
            break
        except Exception:  # noqa: BLE001 - retry transient device errors
            if attempt == 3:
                raise
            time.sleep(20 * (attempt + 1))

    full = np.zeros((B, N, N), np.float32)
    for c in range(NCORES):
        buf = res.results[c]["out"]
        for k, (b, r0, w) in enumerate(assign[c]):
            L = int(lengths[b])
            rows = min(P, L - r0)
            if rows <= 0:
                continue
            full[b, r0:r0 + rows, :L] = (
                buf[k * P:k * P + rows, :L].astype(np.float32)
            )
    return full


# revision 15
# speedup vs baseline: 1.4574x; 1.0535x over previous
"""Trainium2 Bass kernel for nn_AdjacencyMatrixLayer.

Computes, per batch sample b (coordinates x in R^{N x 3}):
    d_ij  = |x_i|^2 - 2 x_i.x_j + |x_j|^2
    A     = exp(-d / sigma^2)
    A     = softmax(A, axis=2) * mask
    out   = A / (sum_j A + 1e-20)

Key structural ideas (v2, on top of the v1 quad kernel):
  * Valid-region truncation: masks are product-of-prefix (valid lengths
    L_b in [N/2, N]); out is zero outside [:L,:L].  Only row-blocks with
    rows < L are computed, at column width W = ceil(L/128)*128, cutting
    ~45% of all engine + DMA work (sum L^2 / (B*N^2) ~ 0.51).
  * Block-major SPMD packing: the work unit is a [128, W] row-block.
    All 8 cores execute ONE identical width-schedule (widths padded so
    each bucket count is divisible by 8); which (sample, row-range) a
    block holds is pure per-core DATA (stationary/moving slices packed
    host-side), so load balance is near-perfect regardless of lengths.
  * One K=22 bf16 matmul per block produces y = -d/sigma^2 - C*(1-v_i*v_j):
    20 hi/lo-split augmented coordinate rows (exact to ~2^-18) + 2 rows
    folding the padding mask (C=144), so masked entries get y - 144.
  * Per block, one of two pointwise schemes, greedily mixed to balance
    the scalar (ACT) and vector (DVE) engines:
      Q (quad):  p = Exp(s*y) on ACT; t = (p+a)*p + accum on DVE stt
                 (1x); out = (t+b)*r on DVE ts (4x).  Minimax quadratic
                 q = p^2+a*p+b ~= K*exp(exp(y)), rel err 5.1e-3.
      E (exact): A = Exp(y) on ACT; q = Exp(A) + accum on ACT;
                 out = q*r on DVE ts (4x).  Exact double exponential;
                 masked entries give A=0, q=1, corrected via the
                 host-provided per-block constant.
    Row renormalization r = 1/(accum + cons) makes the overall scale
    exact; host zero-fills outside [:L,:L].
  * All inputs preloaded in 3 DMAs; output DMA alternates SWDGE/HWDGE.
"""

import math
import sys

import numpy as np

for _p in ("/opt/trn_rl_repo", "/root/.axon_site/_ro/trn_rl_repo"):
    if _p not in sys.path:
        sys.path.append(_p)

B, N, D = 16, 2048, 3
NCORES = 8
P = 128            # SBUF partitions / rows per block
MMF = 512          # matmul moving free-dim chunk (= 1 PSUM bank of fp32)
KAUG = 22          # 20 hi/lo aug rows + 2 mask-fold rows
MASKC = 144.0      # mask fold offset: masked entries get y - 144

# minimax fit of (p^2 + a*p + b) / (K * exp(exp(y))) - 1 over y <= 0
QS_S = 0.9943403856229558   # p = exp(QS_S * y)
QS_A = 1.05888673672267     # q = p^2 + QS_A*p + QS_B
QS_B = 1.217950642291432

# engine-time model (ns per moving column / fixed ns per block), measured
# from perfetto traces of this kernel (includes semaphore overheads)
ACT_NS = 1.004      # one ACT pass over [128, W]
DVE_STT_NS = 1.139  # DVE scalar_tensor_tensor (1x)
DVE_TS_NS = 0.401   # DVE tensor_scalar (4x)
Q_ACT_FIX = 600.0   # per-block fixed engine time (instr + semaphores)
Q_DVE_FIX = 1100.0
E_ACT_FIX = 1180.0
E_DVE_FIX = 800.0
SWDGE_NSPB = 1.0 / 265.0  # ns per byte, measured output queue rates
HWDGE_NSPB = 1.0 / 173.0

_CACHE: dict = {}


def _schedule(lengths):
    """Build the common width schedule + per-core block assignment.

    Returns (widths, schemes, assign) where widths[k] is slot k's moving
    width (same for every core), schemes[k] in {"Q", "E"}, and
    assign[c][k] = (sample, row0, width) for core c slot k (dummy slots
    duplicate a real block; their output is ignored).
    """
    blocks = []  # (L, sample, row0)
    for b, L in enumerate(lengths):
        nb = (int(L) + P - 1) // P
        for r in range(nb):
            blocks.append((int(L), b, r * P))
    # sort by L so each slot's 8 blocks have near-equal lengths, then the
    # slot width (max L in the group, 32-aligned) wastes almost nothing
    blocks.sort(key=lambda x: (-x[0], x[1], x[2]))
    while len(blocks) % NCORES:
        blocks.append(blocks[-1])  # dummy duplicate; output ignored

    widths, assign = [], [[] for _ in range(NCORES)]
    for j in range(len(blocks) // NCORES):
        grp = blocks[j * NCORES:(j + 1) * NCORES]
        w = -(-max(g[0] for g in grp) // 32) * 32
        widths.append(w)
        for c in range(NCORES):
            assign[c].append((grp[c][1], grp[c][2], w))

    # put one narrowest block first so the pipeline starts on a small
    # input chunk (rest stays widest-first, ending narrow for the drain)
    order = list(range(len(widths)))
    order = [order[-1]] + order[:-1]
    widths = [widths[i] for i in order]
    for c in range(NCORES):
        assign[c] = [assign[c][i] for i in order]

    # greedy ACT/DVE balance: scheme per slot
    schemes = []
    act_t = dve_t = 0.0
    for w in widths:
        aq = act_t + ACT_NS * w + Q_ACT_FIX
        dq = dve_t + (DVE_STT_NS + DVE_TS_NS) * w + Q_DVE_FIX
        ae = act_t + 2 * ACT_NS * w + E_ACT_FIX
        de = dve_t + DVE_TS_NS * w + E_DVE_FIX
        if max(ae, de) < max(aq, dq):
            schemes.append("E")
            act_t, dve_t = ae, de
        else:
            schemes.append("Q")
            act_t, dve_t = aq, dq
    return widths, schemes, assign


def _build(widths, schemes):
    import concourse.bacc as bacc
    import concourse.tile as tile
    from concourse import mybir

    f32 = mybir.dt.float32
    f16 = mybir.dt.float16
    bf16 = mybir.dt.bfloat16
    AT = mybir.AluOpType
    AF = mybir.ActivationFunctionType
    nc = bacc.Bacc(None, target_bir_lowering=False, debug=False)

    nblk = len(widths)
    sumw = sum(widths)
    offs = np.concatenate([[0], np.cumsum(widths)]).tolist()

    stat = nc.dram_tensor("stat", [KAUG, nblk * P], bf16, kind="ExternalInput")
    mov = nc.dram_tensor("mov", [KAUG, sumw], bf16, kind="ExternalInput")
    cons = nc.dram_tensor("cons", [P, nblk], f32, kind="ExternalInput")
    out = nc.dram_tensor("out", [nblk * P, N], f16, kind="ExternalOutput")

    with tile.TileContext(nc) as tc:
        with (
            tc.tile_pool(name="consts", bufs=1) as consts,
            tc.tile_pool(name="work", bufs=10) as workp,
            tc.tile_pool(name="ot", bufs=8) as otp,
            tc.tile_pool(name="small", bufs=16) as smallp,
            tc.tile_pool(name="psum", bufs=2, space="PSUM") as psump,
        ):
            st_all = consts.tile([KAUG, nblk * P], bf16, tag="st")
            cn_all = consts.tile([P, nblk], f32, tag="cn")
            nc.sync.dma_start(out=st_all, in_=stat[:, :])
            nc.gpsimd.dma_start(out=cn_all, in_=cons[:, :])

            # moving data preloaded in block-aligned chunks spread over
            # three DMA queues so compute can start after the first chunk
            chunks = [(0, 1)]
            rest = nblk - 1
            ngrp = min(5, rest) or 1
            b = 1
            for g in range(ngrp):
                n = rest // ngrp + (1 if g < rest % ngrp else 0)
                if n:
                    chunks.append((b, b + n))
                    b += n
            chunk_engs = [nc.sync, nc.gpsimd, nc.scalar]
            mv_tiles, mv_of = [], {}
            for ci, (b0, b1) in enumerate(chunks):
                b1 = min(b1, nblk)
                o0, o1 = offs[b0], offs[b1]
                mt = consts.tile([KAUG, o1 - o0], bf16, tag=f"mv{ci}")
                chunk_engs[ci % 3].dma_start(out=mt, in_=mov[:, o0:o1])
                mv_tiles.append(mt)
                for k in range(b0, b1):
                    mv_of[k] = (ci, offs[k] - o0)

            swdge_ns = hwdge_ns = 0.0
            for k in range(nblk):
                W = widths[k]
                st = st_all[:, k * P:(k + 1) * P]
                ci, lo = mv_of[k]
                mv = mv_tiles[ci][:, lo:lo + W]

                ps = psump.tile([P, N], f32)
                for c0 in range(0, W, MMF):
                    cw = min(MMF, W - c0)
                    nc.tensor.matmul(
                        ps[:, c0:c0 + cw], st, mv[:, c0:c0 + cw]
                    )

                qs = smallp.tile([P, 1], f32, tag="qs")
                t = workp.tile([P, N], f16, tag="t")
                if schemes[k] == "Q":
                    # p = exp(s*y); t = (p + a)*p ; qs = sum_j t
                    p = workp.tile([P, N], f16, tag="p")
                    nc.scalar.activation(
                        p[:, :W], ps[:, :W], AF.Exp, scale=QS_S
                    )
                    nc.vector.scalar_tensor_tensor(
                        out=t[:, :W], in0=p[:, :W], scalar=QS_A, in1=p[:, :W],
                        op0=AT.add, op1=AT.mult, accum_out=qs,
                    )
                    cfin = QS_B
                else:
                    # A = exp(y); t = exp(A) ; qs = sum_j t   (exact)
                    a_t = workp.tile([P, N], f16, tag="p")
                    nc.scalar.activation(a_t[:, :W], ps[:, :W], AF.Exp)
                    nc.scalar.activation(
                        t[:, :W], a_t[:, :W], AF.Exp, accum_out=qs
                    )
                    cfin = 0.0

                # qs2 = qs + cons ; r = 1/qs2   (tiny [P,1] ops)
                qs2 = smallp.tile([P, 1], f32, tag="qs2")
                nc.gpsimd.tensor_tensor(
                    out=qs2, in0=qs, in1=cn_all[:, k:k + 1], op=AT.add
                )
                r = smallp.tile([P, 1], f32, tag="r")
                nc.vector.reciprocal(r, qs2)

                # out = (t + cfin) * r   on DVE ts (4x)
                ot = otp.tile([P, N], f16, tag="ot")
                nc.vector.tensor_scalar(
                    out=ot[:, :W], in0=t[:, :W], scalar1=cfin, scalar2=r,
                    op0=AT.add, op1=AT.mult,
                )
                nbytes = W * P * 2
                if swdge_ns + nbytes * SWDGE_NSPB <= hwdge_ns + nbytes * HWDGE_NSPB:
                    swdge_ns += nbytes * SWDGE_NSPB
                    out_eng = nc.gpsimd
                else:
                    hwdge_ns += nbytes * HWDGE_NSPB
                    out_eng = nc.sync
                out_eng.dma_start(
                    out=out[k * P:(k + 1) * P, :W], in_=ot[:, :W]
                )

    nc.compile()
    return nc


def _lengths_from_masks(masks):
    """Per-sample valid lengths; verifies the product-prefix structure."""
    diag = np.einsum('bii->bi', masks)
    valid = (diag > 0.5).astype(np.float32)
    lengths = valid.sum(axis=1).astype(np.int64)
    n = masks.shape[1]
    pref = (np.arange(n)[None, :] < lengths[:, None]).astype(np.float32)
    if not np.array_equal(valid, pref):
        return None
    if not np.array_equal(masks, valid[:, :, None] * valid[:, None, :]):
        return None
    return lengths, valid


def _prepare(coordinates, masks, sigma):
    """Host-side prep: schedule blocks, pack per-core block-major inputs."""
    import ml_dtypes

    bf = ml_dtypes.bfloat16
    coords = np.ascontiguousarray(np.asarray(coordinates, dtype=np.float32))
    masks = np.asarray(masks, dtype=np.float32)
    sig = float(np.asarray(sigma, dtype=np.float32).reshape(-1)[0])

    res = _lengths_from_masks(masks)
    assert res is not None, "masks are not product-of-prefix form"
    lengths, valid = res
    widths, schemes, assign = _schedule(lengths)
    nblk = len(widths)
    sumw = sum(widths)
    offs = np.concatenate([[0], np.cumsum(widths)])

    norms = np.sum(coords * coords, axis=2, dtype=np.float32)  # [B, N]
    xT = np.swapaxes(coords, 1, 2)                             # [B, 3, N]
    nss = np.float32(-1.0 / (sig * sig))
    aug_x = np.empty((B, 5, N), np.float32)
    aug_x[:, 0:3] = (-2.0 * nss) * xT
    aug_x[:, 3] = nss * norms
    aug_x[:, 4] = nss
    aug_y = np.empty((B, 5, N), np.float32)
    aug_y[:, 0:3] = xT
    aug_y[:, 3] = 1.0
    aug_y[:, 4] = norms

    # hi/lo bf16 split: v = hi + lo, K=5 fp32 -> K=20 bf16 contraction
    xh = aug_x.astype(bf)
    xl = (aug_x - xh.astype(np.float32)).astype(bf)
    yh = aug_y.astype(bf)
    yl = (aug_y - yh.astype(np.float32)).astype(bf)
    # mask fold rows: C*v_i*v_j - C  (exact in bf16: C=144, v in {0,1})
    C = np.float32(MASKC)
    mx = np.stack([C * valid, np.full_like(valid, C)], axis=1).astype(bf)
    my = np.stack([valid, np.full_like(valid, -1.0)], axis=1).astype(bf)
    augx22 = np.concatenate([xh, xl, xh, xl, mx], axis=1)  # [B, 22, N]
    augy22 = np.concatenate([yh, yh, yl, yl, my], axis=1)

    in_maps = []
    for c in range(NCORES):
        stat = np.empty((KAUG, nblk * P), bf)
        mov = np.empty((KAUG, sumw), bf)
        cons = np.empty((P, nblk), np.float32)
        for k, (b, r0, w) in enumerate(assign[c]):
            stat[:, k * P:(k + 1) * P] = augx22[b][:, r0:r0 + P]
            mov[:, offs[k]:offs[k] + w] = augy22[b][:, :w]
            L = float(lengths[b])
            cons[:, k] = QS_B * L if schemes[k] == "Q" else -(w - L)
        in_maps.append({"stat": stat, "mov": mov, "cons": cons})
    return in_maps, (lengths, widths, schemes, assign)


def _get_nc(widths=None, schemes=None):
    if "nc" not in _CACHE:
        _CACHE["nc"] = _build(widths, schemes)
    return _CACHE["nc"]


def kernel(coordinates, masks, sigma):
    import time

    from concourse.bass_utils import run_bass_kernel_spmd

    in_maps, (lengths, widths, schemes, assign) = _prepare(
        coordinates, masks, sigma
    )
    nc = _get_nc(widths, schemes)
    # the shared trn2 device occasionally reports a transient
    # NRT_EXEC_UNIT_UNRECOVERABLE; it clears on its own within ~a minute
    for attempt in range(4):
        try:
            res = run_bass_kernel_spmd(
                nc, in_maps, core_ids=list(range(NCORES))
            )
            break
        except Exception:  # noqa: BLE001 - retry transient device errors
            if attempt == 3:
                raise
            time.sleep(20 * (attempt + 1))

    full = np.zeros((B, N, N), np.float32)
    for c in range(NCORES):
        buf = res.results[c]["out"]
        for k, (b, r0, w) in enumerate(assign[c]):
            L = int(lengths[b])
            rows = min(P, L - r0)
            if rows <= 0:
                continue
            full[b, r0:r0 + rows, :L] = (
                buf[k * P:k * P + rows, :L].astype(np.float32)
            )
    return full


# revision 18
# speedup vs baseline: 1.4643x; 1.0047x over previous
"""Trainium2 Bass kernel for nn_AdjacencyMatrixLayer.

Computes, per batch sample b (coordinates x in R^{N x 3}):
    d_ij  = |x_i|^2 - 2 x_i.x_j + |x_j|^2
    A     = exp(-d / sigma^2)
    A     = softmax(A, axis=2) * mask
    out   = A / (sum_j A + 1e-20)

Key structural ideas (v2, on top of the v1 quad kernel):
  * Valid-region truncation: masks are product-of-prefix (valid lengths
    L_b in [N/2, N]); out is zero outside [:L,:L].  Only row-blocks with
    rows < L are computed, at column width W = ceil(L/128)*128, cutting
    ~45% of all engine + DMA work (sum L^2 / (B*N^2) ~ 0.51).
  * Block-major SPMD packing: the work unit is a [128, W] row-block.
    All 8 cores execute ONE identical width-schedule (widths padded so
    each bucket count is divisible by 8); which (sample, row-range) a
    block holds is pure per-core DATA (stationary/moving slices packed
    host-side), so load balance is near-perfect regardless of lengths.
  * One K=22 bf16 matmul per block produces y = -d/sigma^2 - C*(1-v_i*v_j):
    20 hi/lo-split augmented coordinate rows (exact to ~2^-18) + 2 rows
    folding the padding mask (C=144), so masked entries get y - 144.
  * Per block, one of two pointwise schemes, greedily mixed to balance
    the scalar (ACT) and vector (DVE) engines:
      Q (quad):  p = Exp(s*y) on ACT; t = (p+a)*p + accum on DVE stt
                 (1x); out = (t+b)*r on DVE ts (4x).  Minimax quadratic
                 q = p^2+a*p+b ~= K*exp(exp(y)), rel err 5.1e-3.
      E (exact): A = Exp(y) on ACT; q = Exp(A) + accum on ACT;
                 out = q*r on DVE ts (4x).  Exact double exponential;
                 masked entries give A=0, q=1, corrected via the
                 host-provided per-block constant.
    Row renormalization r = 1/(accum + cons) makes the overall scale
    exact; host zero-fills outside [:L,:L].
  * All inputs preloaded in 3 DMAs; output DMA alternates SWDGE/HWDGE.
"""

import math
import sys

import numpy as np

for _p in ("/opt/trn_rl_repo", "/root/.axon_site/_ro/trn_rl_repo"):
    if _p not in sys.path:
        sys.path.append(_p)

B, N, D = 16, 2048, 3
NCORES = 8
P = 128            # SBUF partitions / rows per block
MMF = 512          # matmul moving free-dim chunk (= 1 PSUM bank of fp32)
KAUG = 22          # 20 hi/lo aug rows + 2 mask-fold rows
MASKC = 144.0      # mask fold offset: masked entries get y - 144

# minimax fit of (p^2 + a*p + b) / (K * exp(exp(y))) - 1 over y <= 0
QS_S = 0.9943403856229558   # p = exp(QS_S * y)
QS_A = 1.05888673672267     # q = p^2 + QS_A*p + QS_B
QS_B = 1.217950642291432

# engine-time model (ns per moving column / fixed ns per block), measured
# from perfetto traces of this kernel (includes semaphore overheads)
ACT_NS = 1.004      # one ACT pass over [128, W]
DVE_STT_NS = 1.139  # DVE scalar_tensor_tensor (1x)
DVE_TS_NS = 0.401   # DVE tensor_scalar (4x)
Q_ACT_FIX = 600.0   # per-block fixed engine time (instr + semaphores)
Q_DVE_FIX = 1100.0
E_ACT_FIX = 1180.0
E_DVE_FIX = 800.0
SWDGE_NSPB = 1.0  # output queue byte-balance weights (50/50)
HWDGE_NSPB = 1.0
DVE_TAIL_BIAS = 3000.0  # make DVE's queue drain ~with ACT's at the end
FORCE_E_TAIL = 3        # last slots DVE-light so the drain is short

_CACHE: dict = {}


def _schedule(lengths):
    """Build the common width schedule + per-core block assignment.

    Returns (widths, schemes, assign) where widths[k] is slot k's moving
    width (same for every core), schemes[k] in {"Q", "E"}, and
    assign[c][k] = (sample, row0, width) for core c slot k (dummy slots
    duplicate a real block; their output is ignored).
    """
    blocks = []  # (L, sample, row0)
    for b, L in enumerate(lengths):
        nb = (int(L) + P - 1) // P
        for r in range(nb):
            blocks.append((int(L), b, r * P))
    # sort by L so each slot's 8 blocks have near-equal lengths, then the
    # slot width (max L in the group, 32-aligned) wastes almost nothing
    blocks.sort(key=lambda x: (-x[0], x[1], x[2]))
    while len(blocks) % NCORES:
        blocks.append(blocks[-1])  # dummy duplicate; output ignored

    widths, assign = [], [[] for _ in range(NCORES)]
    for j in range(len(blocks) // NCORES):
        grp = blocks[j * NCORES:(j + 1) * NCORES]
        w = -(-max(g[0] for g in grp) // 32) * 32
        widths.append(w)
        for c in range(NCORES):
            assign[c].append((grp[c][1], grp[c][2], w))

    # put one narrowest block first so the pipeline starts on a small
    # input chunk (rest stays widest-first, ending narrow for the drain)
    order = list(range(len(widths)))
    order = [order[-1]] + order[:-1]
    widths = [widths[i] for i in order]
    for c in range(NCORES):
        assign[c] = [assign[c][i] for i in order]

    # greedy ACT/DVE balance: scheme per slot (biased so DVE's queue,
    # which depends on ACT's output, drains at the same time as ACT's)
    schemes = []
    act_t = dve_t = 0.0
    for k, w in enumerate(widths):
        aq = act_t + ACT_NS * w + Q_ACT_FIX
        dq = dve_t + (DVE_STT_NS + DVE_TS_NS) * w + Q_DVE_FIX
        ae = act_t + 2 * ACT_NS * w + E_ACT_FIX
        de = dve_t + DVE_TS_NS * w + E_DVE_FIX
        force_e = k >= len(widths) - FORCE_E_TAIL
        if force_e or max(ae, de + DVE_TAIL_BIAS) < max(aq, dq + DVE_TAIL_BIAS):
            schemes.append("E")
            act_t, dve_t = ae, de
        else:
            schemes.append("Q")
            act_t, dve_t = aq, dq
    return widths, schemes, assign


def _build(widths, schemes):
    import concourse.bacc as bacc
    import concourse.tile as tile
    from concourse import mybir

    f32 = mybir.dt.float32
    f16 = mybir.dt.float16
    bf16 = mybir.dt.bfloat16
    AT = mybir.AluOpType
    AF = mybir.ActivationFunctionType
    nc = bacc.Bacc(None, target_bir_lowering=False, debug=False)

    nblk = len(widths)
    sumw = sum(widths)
    offs = np.concatenate([[0], np.cumsum(widths)]).tolist()

    stat = nc.dram_tensor("stat", [KAUG, nblk * P], bf16, kind="ExternalInput")
    mov = nc.dram_tensor("mov", [KAUG, sumw], bf16, kind="ExternalInput")
    cons = nc.dram_tensor("cons", [P, nblk], f32, kind="ExternalInput")
    out = nc.dram_tensor("out", [nblk * P, N], f16, kind="ExternalOutput")

    with tile.TileContext(nc) as tc:
        with (
            tc.tile_pool(name="consts", bufs=1) as consts,
            tc.tile_pool(name="work", bufs=10) as workp,
            tc.tile_pool(name="ot", bufs=8) as otp,
            tc.tile_pool(name="small", bufs=16) as smallp,
            tc.tile_pool(name="psum", bufs=2, space="PSUM") as psump,
        ):
            st_all = consts.tile([KAUG, nblk * P], bf16, tag="st")
            cn_all = consts.tile([P, nblk], f32, tag="cn")

            # moving data preloaded in block-aligned chunks spread over
            # three DMA queues so compute can start after the first chunk;
            # slot 0's chunk goes first, before the stationary preload
            chunks = [(0, 1)]
            rest = nblk - 1
            ngrp = min(5, rest) or 1
            b = 1
            for g in range(ngrp):
                n = rest // ngrp + (1 if g < rest % ngrp else 0)
                if n:
                    chunks.append((b, b + n))
                    b += n
            chunk_engs = [nc.sync, nc.gpsimd, nc.scalar]
            mv_tiles, mv_of = [], {}
            for ci, (b0, b1) in enumerate(chunks):
                b1 = min(b1, nblk)
                o0, o1 = offs[b0], offs[b1]
                mt = consts.tile([KAUG, o1 - o0], bf16, tag=f"mv{ci}")
                chunk_engs[ci % 3].dma_start(out=mt, in_=mov[:, o0:o1])
                mv_tiles.append(mt)
                for k in range(b0, b1):
                    mv_of[k] = (ci, offs[k] - o0)
                if ci == 0:
                    nc.sync.dma_start(out=st_all, in_=stat[:, :])
                    nc.gpsimd.dma_start(out=cn_all, in_=cons[:, :])

            swdge_ns = hwdge_ns = 0.0
            for k in range(nblk):
                W = widths[k]
                st = st_all[:, k * P:(k + 1) * P]
                ci, lo = mv_of[k]
                mv = mv_tiles[ci][:, lo:lo + W]

                ps = psump.tile([P, N], f32)
                for c0 in range(0, W, MMF):
                    cw = min(MMF, W - c0)
                    nc.tensor.matmul(
                        ps[:, c0:c0 + cw], st, mv[:, c0:c0 + cw]
                    )

                qs = smallp.tile([P, 1], f32, tag="qs")
                t = workp.tile([P, N], f16, tag="t")
                if schemes[k] == "Q":
                    # p = exp(s*y); t = (p + a)*p ; qs = sum_j t
                    p = workp.tile([P, N], f16, tag="p")
                    nc.scalar.activation(
                        p[:, :W], ps[:, :W], AF.Exp, scale=QS_S
                    )
                    nc.vector.scalar_tensor_tensor(
                        out=t[:, :W], in0=p[:, :W], scalar=QS_A, in1=p[:, :W],
                        op0=AT.add, op1=AT.mult, accum_out=qs,
                    )
                    cfin = QS_B
                else:
                    # A = exp(y); t = exp(A) ; qs = sum_j t   (exact)
                    a_t = workp.tile([P, N], f16, tag="p")
                    nc.scalar.activation(a_t[:, :W], ps[:, :W], AF.Exp)
                    nc.scalar.activation(
                        t[:, :W], a_t[:, :W], AF.Exp, accum_out=qs
                    )
                    cfin = 0.0

                # qs2 = qs + cons ; r = 1/qs2   (tiny [P,1] ops)
                qs2 = smallp.tile([P, 1], f32, tag="qs2")
                nc.gpsimd.tensor_tensor(
                    out=qs2, in0=qs, in1=cn_all[:, k:k + 1], op=AT.add
                )
                r = smallp.tile([P, 1], f32, tag="r")
                nc.vector.reciprocal(r, qs2)

                # out = (t + cfin) * r   on DVE ts (4x)
                ot = otp.tile([P, N], f16, tag="ot")
                nc.vector.tensor_scalar(
                    out=ot[:, :W], in0=t[:, :W], scalar1=cfin, scalar2=r,
                    op0=AT.add, op1=AT.mult,
                )
                nbytes = W * P * 2
                if swdge_ns + nbytes * SWDGE_NSPB <= hwdge_ns + nbytes * HWDGE_NSPB:
                    swdge_ns += nbytes * SWDGE_NSPB
                    out_eng = nc.gpsimd
                else:
                    hwdge_ns += nbytes * HWDGE_NSPB
                    out_eng = nc.sync
                out_eng.dma_start(
                    out=out[k * P:(k + 1) * P, :W], in_=ot[:, :W]
                )

    nc.compile()
    return nc


def _lengths_from_masks(masks):
    """Per-sample valid lengths; verifies the product-prefix structure."""
    diag = np.einsum('bii->bi', masks)
    valid = (diag > 0.5).astype(np.float32)
    lengths = valid.sum(axis=1).astype(np.int64)
    n = masks.shape[1]
    pref = (np.arange(n)[None, :] < lengths[:, None]).astype(np.float32)
    if not np.array_equal(valid, pref):
        return None
    if not np.array_equal(masks, valid[:, :, None] * valid[:, None, :]):
        return None
    return lengths, valid


def _prepare(coordinates, masks, sigma):
    """Host-side prep: schedule blocks, pack per-core block-major inputs."""
    import ml_dtypes

    bf = ml_dtypes.bfloat16
    coords = np.ascontiguousarray(np.asarray(coordinates, dtype=np.float32))
    masks = np.asarray(masks, dtype=np.float32)
    sig = float(np.asarray(sigma, dtype=np.float32).reshape(-1)[0])

    res = _lengths_from_masks(masks)
    assert res is not None, "masks are not product-of-prefix form"
    lengths, valid = res
    widths, schemes, assign = _schedule(lengths)
    nblk = len(widths)
    sumw = sum(widths)
    offs = np.concatenate([[0], np.cumsum(widths)])

    norms = np.sum(coords * coords, axis=2, dtype=np.float32)  # [B, N]
    xT = np.swapaxes(coords, 1, 2)                             # [B, 3, N]
    nss = np.float32(-1.0 / (sig * sig))
    aug_x = np.empty((B, 5, N), np.float32)
    aug_x[:, 0:3] = (-2.0 * nss) * xT
    aug_x[:, 3] = nss * norms
    aug_x[:, 4] = nss
    aug_y = np.empty((B, 5, N), np.float32)
    aug_y[:, 0:3] = xT
    aug_y[:, 3] = 1.0
    aug_y[:, 4] = norms

    # hi/lo bf16 split: v = hi + lo, K=5 fp32 -> K=20 bf16 contraction
    xh = aug_x.astype(bf)
    xl = (aug_x - xh.astype(np.float32)).astype(bf)
    yh = aug_y.astype(bf)
    yl = (aug_y - yh.astype(np.float32)).astype(bf)
    # mask fold rows: C*v_i*v_j - C  (exact in bf16: C=144, v in {0,1})
    C = np.float32(MASKC)
    mx = np.stack([C * valid, np.full_like(valid, C)], axis=1).astype(bf)
    my = np.stack([valid, np.full_like(valid, -1.0)], axis=1).astype(bf)
    augx22 = np.concatenate([xh, xl, xh, xl, mx], axis=1)  # [B, 22, N]
    augy22 = np.concatenate([yh, yh, yl, yl, my], axis=1)

    in_maps = []
    for c in range(NCORES):
        stat = np.empty((KAUG, nblk * P), bf)
        mov = np.empty((KAUG, sumw), bf)
        cons = np.empty((P, nblk), np.float32)
        for k, (b, r0, w) in enumerate(assign[c]):
            stat[:, k * P:(k + 1) * P] = augx22[b][:, r0:r0 + P]
            mov[:, offs[k]:offs[k] + w] = augy22[b][:, :w]
            L = float(lengths[b])
            cons[:, k] = QS_B * L if schemes[k] == "Q" else -(w - L)
        in_maps.append({"stat": stat, "mov": mov, "cons": cons})
    return in_maps, (lengths, widths, schemes, assign)


def _get_nc(widths=None, schemes=None):
    if "nc" not in _CACHE:
        _CACHE["nc"] = _build(widths, schemes)
    return _CACHE["nc"]


def kernel(coordinates, masks, sigma):
    import time

    from concourse.bass_utils import run_bass_kernel_spmd

    in_maps, (lengths, widths, schemes, assign) = _prepare(
        coordinates, masks, sigma
    )
    nc = _get_nc(widths, schemes)
    # the shared trn2 device occasionally reports a transient
    # NRT_EXEC_UNIT_UNRECOVERABLE; it clears on its own within ~a minute
    for attempt in range(4):
        try:
            res = run_bass_kernel_spmd(
                nc, in_maps, core_ids=list(range(NCORES))
            )
            break
        except Exception:  # noqa: BLE001 - retry transient device errors
            if attempt == 3:
                raise
            time.sleep(20 * (attempt + 1))

    full = np.zeros((B, N, N), np.float32)
    for c in range(NCORES):
        buf = res.results[c]["out"]
        for k, (b, r0, w) in enumerate(assign[c]):
            L = int(lengths[b])
            rows = min(P, L - r0)
            if rows <= 0:
                continue
            full[b, r0:r0 + rows, :L] = (
                buf[k * P:k * P + rows, :L].astype(np.float32)
            )
    return full
